# revision 16
# baseline (speedup 1.0000x reference)
"""Trainium2 Bass kernel: single-channel 2D conv (valid), X[8192,8192] * w[5,5] + bias.

Row-shard the first 7936 output rows across 8 NeuronCores (8 full
124-row matmul bands per core, 992 rows each, with a 4-row halo); the
remaining 252-row bottom strip is sharded column-wise, each core taking a
[256 x 1032] patch. This keeps every TensorE matmul a full-size
K=128/M=124/N=512 stream (the leftover rows cost 30 small-strip matmuls
per core instead of an 80-matmul 9th band).

Per output tile, the conv runs as 5 PSUM-accumulated TensorE matmuls: for
each kernel column dj, a banded stationary A_dj[k, m] = w[k-m, dj]
(0 <= k-m < 5) contracts over 128 input rows to produce 124 output rows of
the column-direction conv, with the moving operand the input tile shifted
by dj columns.

I/O is bf16 on the wire (host casts; rel-err ~3e-3 vs the 2e-2 gate).
Bulk loads and mid-kernel stores ride single whole-band SWDGE DMAs (the
partition swizzle spreads them across all 16 SDMA engines); the final
stores ride the two HWDGE rings because the gpsimd end-of-kernel DRAIN
costs ~11us after its queue's last transfer completes. A burst of warm-up
matmuls flips the PE HAM clock gate to 2.4 GHz before the first tile lands.
"""

import ml_dtypes
import numpy as np

import concourse.bass as bass
import concourse.mybir as mybir
from concourse import bacc
from concourse import bass_utils
from concourse.tile import TileContext

H = 8192
W = 8192
KH = 5
KW = 5
OH = H - KH + 1  # 8188
OW = W - KW + 1  # 8188

NCORES = 8
BAND_OUT = 124  # output rows per matmul band (K=128 partitions -> M=124)
BAND_IN = BAND_OUT + KH - 1  # 128
NBANDS = 8
ROWS_MAIN = NBANDS * BAND_OUT  # 992 output rows per core
MAIN_IN = ROWS_MAIN + KH - 1  # 996 input rows per core
SUB_W = 512  # matmul moving free dim (one PSUM bank of fp32)

# Bottom strip: output rows [7936, 8188), each core takes 1024 output cols.
STRIP_R0 = NCORES * ROWS_MAIN  # 7936
STRIP_ROWS = OH - STRIP_R0  # 252
STRIP_IN_ROWS = STRIP_ROWS + KH - 1  # 256
STRIP_W = 1024  # output cols per core
STRIP_IN_W = STRIP_W + 4  # 1028: +4 conv halo
# Strip bands: (band_row0, store_lo) — band 2 overlaps band 1, storing only
# its last 4 rows.
_STRIP_BANDS = [(0, 0), (124, 0), (STRIP_ROWS - BAND_OUT, 120)]
_STRIP_SUBS = [0, 512]  # two exact 512-wide subtiles cover the 1024 cols

_SUB_STARTS = [512 * i for i in range(15)] + [OW - SUB_W]

_PROGRAM_CACHE = {}

TRACE = False
LAST_RUN = {}

BF16 = ml_dtypes.bfloat16


def _load_splits(bi):
    # Column split points for a main band's input load; subtile s reads
    # columns [512s, 512s + 516), so splits at 512s + 4 keep each subtile's
    # dependency to the minimal set of pieces.
    if bi == 0:
        return [0, 516, 1028, 2052, 4100, W]
    return [0, 4100, W]


def _build_program(bias_val: float):
    f32 = mybir.dt.float32
    bf16 = mybir.dt.bfloat16

    nc = bacc.Bacc("TRN2", target_bir_lowering=False, debug=False, num_devices=NCORES)

    Xs = nc.dram_tensor("Xs", [MAIN_IN, W], bf16, kind="ExternalInput")
    Xt = nc.dram_tensor("Xt", [STRIP_IN_ROWS, STRIP_IN_W], bf16, kind="ExternalInput")
    Aw = nc.dram_tensor("Aw", [128, KW * BAND_OUT], bf16, kind="ExternalInput")
    # Row-padded outputs; host crops (8192 -> 8188 cols, 1028 -> 1024 cols).
    Y = nc.dram_tensor("Y", [ROWS_MAIN, W], bf16, kind="ExternalOutput")
    Yt = nc.dram_tensor("Yt", [STRIP_ROWS, STRIP_W], bf16, kind="ExternalOutput")

    def copy_out(dst, src, si):
        if bias_val == 0.0:
            if si % 2 == 0:
                nc.vector.tensor_copy(dst, src)
            else:
                nc.scalar.activation(dst, src, mybir.ActivationFunctionType.Copy)
        else:
            nc.scalar.activation(
                dst, src, mybir.ActivationFunctionType.Copy, bias=bias_val
            )

    with TileContext(nc) as tc:
        with (
            tc.tile_pool(name="const", bufs=1) as cpool,
            tc.tile_pool(name="inp", bufs=3) as in_pool,
            tc.tile_pool(name="strip_inp", bufs=3) as sin_pool,
            tc.tile_pool(name="outp", bufs=3) as out_pool,
            tc.tile_pool(name="strip_outp", bufs=3) as sout_pool,
            tc.tile_pool(name="psum", bufs=8, space="PSUM") as psum_pool,
        ):
            A_t = cpool.tile([128, KW * BAND_OUT], bf16)
            nc.sync.dma_start(A_t[:], Aw.ap())

            # PE warm-up: ~4us of junk matmuls flips the HAM clock gate to
            # 8/8 (2.4 GHz) before the first input tile lands. Results land
            # in rotating PSUM banks and are overwritten by the first real
            # accumulation groups (start=True clears the bank).
            for _ in range(10):
                wps = psum_pool.tile([BAND_OUT, SUB_W], f32, name="ps")
                nc.tensor.matmul(
                    wps[:], A_t[0:128, 0:BAND_OUT], A_t[0:128, 0:SUB_W],
                    start=True, stop=True,
                )

            def load_band(bi):
                r0 = BAND_OUT * bi
                in_t = in_pool.tile([BAND_IN, W], bf16, name="in_t")
                splits = _load_splits(bi)
                for c0, c1 in zip(splits, splits[1:]):
                    nc.gpsimd.dma_start(
                        in_t[:, c0:c1], Xs.ap()[r0 : r0 + BAND_IN, c0:c1]
                    )
                return in_t

            # Loads run two bands ahead of the store chunks in the SWDGE
            # FIFO so a store's copy-wait never delays a load's issue.
            in_tiles = {0: load_band(0), 1: load_band(1)}
            strip_tiles = []
            for sr0, _ in _STRIP_BANDS:
                st = sin_pool.tile([BAND_IN, STRIP_IN_W], bf16, name="st")
                nc.gpsimd.dma_start(st[:], Xt.ap()[sr0 : sr0 + BAND_IN, :])
                strip_tiles.append(st)

            for bi in range(NBANDS):
                r0 = BAND_OUT * bi
                if bi + 2 < NBANDS:
                    in_tiles[bi + 2] = load_band(bi + 2)
                in_t = in_tiles.pop(bi)
                out_t = out_pool.tile([BAND_OUT, W], bf16)
                for si, c0 in enumerate(_SUB_STARTS):
                    ps = psum_pool.tile([BAND_OUT, SUB_W], f32)
                    for dj in range(KW):
                        nc.tensor.matmul(
                            ps[:],
                            A_t[0:BAND_IN, dj * BAND_OUT : (dj + 1) * BAND_OUT],
                            in_t[:, c0 + dj : c0 + dj + SUB_W],
                            start=(dj == 0),
                            stop=(dj == KW - 1),
                        )
                    copy_out(out_t[:, c0 : c0 + SUB_W], ps[:], si)
                # Store eagerly in two column chunks: chunk g only depends on
                # subtiles 8g..8g+7's copies, so the store pipeline stays fed
                # throughout the band instead of bunching 2MB at kernel end
                # (outstanding stores at the end skew the final cross-core
                # barrier).
                for g in range(2):
                    ca, cb = 4096 * g, 4096 * (g + 1)
                    nc.gpsimd.dma_start(
                        Y.ap()[r0 : r0 + BAND_OUT, ca:cb], out_t[:, ca:cb]
                    )

            # Bottom strip: 3 bands x 3 subtiles on this core's column patch.
            # Strip stores ride HWDGE (the gpsimd DRAIN costs ~11us after the
            # last SWDGE transfer completes).
            for sbi, ((sr0, store_lo), st) in enumerate(
                zip(_STRIP_BANDS, strip_tiles)
            ):
                cl = (store_lo // 32) * 32
                so_t = sout_pool.tile([BAND_OUT, STRIP_W], bf16)
                for si, c0 in enumerate(_STRIP_SUBS):
                    ps = psum_pool.tile([BAND_OUT, SUB_W], f32, name="ps")
                    for dj in range(KW):
                        nc.tensor.matmul(
                            ps[:],
                            A_t[0:BAND_IN, dj * BAND_OUT : (dj + 1) * BAND_OUT],
                            st[:, c0 + dj : c0 + dj + SUB_W],
                            start=(dj == 0),
                            stop=(dj == KW - 1),
                        )
                    copy_out(so_t[cl:BAND_OUT, c0 : c0 + SUB_W], ps[cl:BAND_OUT, :], si)
                q = nc.sync if sbi % 2 == 0 else nc.scalar
                q.dma_start(
                    Yt.ap()[sr0 + store_lo : sr0 + BAND_OUT, :],
                    so_t[store_lo:BAND_OUT, :],
                )

    nc.compile()
    return nc


def kernel(X, weight, bias):
    X = np.ascontiguousarray(np.asarray(X, dtype=np.float32))
    weight = np.asarray(weight, dtype=np.float32)
    bias = np.asarray(bias, dtype=np.float32)
    assert X.shape == (H, W) and weight.shape == (KH, KW)

    bias_val = float(bias.reshape(-1)[0])
    nc = _PROGRAM_CACHE.get(bias_val)
    if nc is None:
        nc = _build_program(bias_val)
        _PROGRAM_CACHE[bias_val] = nc

    # Banded stationary matrices: A[k, dj*124 + m] = w[k-m, dj] for 0<=k-m<5
    A = np.zeros((128, KW * BAND_OUT), dtype=np.float32)
    m = np.arange(BAND_OUT)
    for dj in range(KW):
        for di in range(KH):
            A[m + di, dj * BAND_OUT + m] = weight[di, dj]
    A = A.astype(BF16)

    Xb = X.astype(BF16)
    strip = np.zeros((NCORES, STRIP_IN_ROWS, STRIP_IN_W), dtype=BF16)
    for c in range(NCORES):
        c0 = c * STRIP_W
        c1 = min(W, c0 + STRIP_IN_W)
        strip[c, :, : c1 - c0] = Xb[STRIP_R0:H, c0:c1]
    in_maps = [
        {"Xs": Xb[c * ROWS_MAIN : c * ROWS_MAIN + MAIN_IN], "Xt": strip[c], "Aw": A}
        for c in range(NCORES)
    ]

    res = bass_utils.run_bass_kernel_spmd(
        nc, in_maps, core_ids=list(range(NCORES)), trace=TRACE
    )
    LAST_RUN.clear()
    LAST_RUN.update(
        exec_time_ns=res.exec_time_ns,
        instructions_and_trace=res.instructions_and_trace,
        profile_json=res.profile_json,
    )

    out = np.empty((OH, OW), dtype=np.float32)
    main = np.concatenate([res.results[c]["Y"] for c in range(NCORES)], axis=0)
    out[:STRIP_R0] = main[:, :OW].astype(np.float32)
    stripe = np.concatenate(
        [res.results[c]["Yt"][:, :STRIP_W] for c in range(NCORES)], axis=1
    )
    out[STRIP_R0:] = stripe[:, :OW].astype(np.float32)
    return out
